# revision 13
# baseline (speedup 1.0000x reference)
"""Multi-head attention (B=4, N=2048, C=1024, H=16, HD=64) on 8 TRN2 NeuronCores.

Self-contained: takes the FULL unsharded inputs of the reference problem,
shards across 8 cores, runs a Bass/Tile kernel SPMD, and reassembles the
full output.

Sharding (tensor-parallel over heads x data-parallel over batch):
  core i -> batch b = i//2, head-group g = i%2 (8 of the 16 heads).
  Unsharding sums the two partial output projections per batch.

v2 pipeline (per core, heads processed in 4 pairs sharing a 128-row dslot):
  per pair p (head A=2p at partitions 0:64, B=2p+1 at 64:128):
    KTp/QTp [128d, N] and VNp [128k, ko, 2, 65] computed just-in-time;
    pair p+1's QKV matmul groups are sprinkled one-per-slot into pair p's
    32 (half, ko) attention slots so the PE never idles the ScalarE.
    per slot: S^T tiles [128k, 1024q] for A and B (the two heads' K=64
      matmuls land in disjoint PE row-groups via their base partitions);
      exp on ScalarE (scale=1/8, bias=-5 keeps E finite; the constant
      shift cancels in softmax) -> ET bf16; PV lags one ko so the
      in-order PE never waits on the exp it just requested.
      PV keeps V stationary (LDWEIGHTS amortized over 512-wide ET
      streams): psO[65, 1024] += VN_aug.T @ ET -- the ones column
      (row 64) accumulates the softmax denominators.
    norm per (head, half): reciprocal_approx_fast on the den row,
      GPSIMD partition_broadcast down 64 partitions, one DVE multiply
      -> OT[d, n] (already transposed for the projection; no PE
      transposes anywhere).
  out [N, OUTC] = OT.T @ wpT + pb  (partial: this head-group's channels)
"""

import sys

sys.path.insert(0, "/opt/trn_rl_repo")

from contextlib import ExitStack

import numpy as np

import concourse.bass as bass
import concourse.tile as tile
from concourse import mybir
from concourse.bass_utils import run_bass_kernel_spmd

F32 = mybir.dt.float32
BF16 = mybir.dt.bfloat16

B, N, C, H = 4, 2048, 1024, 16
HH = H // 2          # heads per core
HD = C // H          # head dim
DH = HH * HD         # attention channels per core
OUTC = C
NPAIR = HH // 2      # head pairs per core (share a 128-row dslot)
KO = N // 128        # 16 key blocks
SCALE = float(HD) ** -0.5
EXP_BIAS = -5.0      # exp(s*scale + bias); cancels in softmax normalization


def _split_multi_waits(nc, max_waits=1):
    """The pinned walrus build rejects >1 SyncWait on engine CTRL
    instructions; move extra waits onto preceding NOPs."""
    n_split = 0
    for bb in nc.main_func.blocks:
        insts = list(bb.instructions)
        new_insts = []
        changed = False
        for ins in insts:
            si = getattr(ins, "sync_info", None)
            nm = type(ins).__name__
            is_dma = "Dma" in nm or "TensorLoad" in nm or "TensorSave" in nm
            if si is not None and not is_dma:
                waits = list(si.on_wait)
                if len(waits) > max_waits:
                    head, tail = waits[:-max_waits], waits[-max_waits:]
                    for i in range(0, len(head), max_waits):
                        new_insts.append(
                            mybir.InstNoOp(
                                name=f"{ins.name}-ws{n_split}-{i}",
                                engine=ins.engine,
                                sync_info=mybir.SyncInfo(
                                    on_wait=head[i : i + max_waits], on_update=[]
                                ),
                                bass_nofuse=True,
                            )
                        )
                    ins.sync_info = mybir.SyncInfo(
                        on_wait=tail, on_update=list(si.on_update)
                    )
                    n_split += 1
                    changed = True
            new_insts.append(ins)
        if changed:
            bb.instructions = new_insts
    return n_split


def build_attention_nc(R=1):
    CO = C // 128        # 8 contraction blocks for qkv projections
    DO = DH // 128       # 4 dslots
    QH = N // 2          # 1024-query half

    nc = bass.Bass()
    xT_e = nc.declare_dram_parameter("xT", [C, N], F32, isOutput=False)
    wqT_e = nc.declare_dram_parameter("wqT", [C, DH], F32, isOutput=False)
    wkT_e = nc.declare_dram_parameter("wkT", [C, DH], F32, isOutput=False)
    wvT_e = nc.declare_dram_parameter("wvT", [C, DH], F32, isOutput=False)
    wpT_e = nc.declare_dram_parameter("wpT", [DH, OUTC], F32, isOutput=False)
    pb_e = nc.declare_dram_parameter("pb", [1, OUTC], F32, isOutput=False)
    out_e = nc.declare_dram_parameter("out", [N, OUTC], F32, isOutput=True)

    with tile.TileContext(nc) as tc:

        def body(_iv=None):
            with ExitStack() as ctx:
                persist = ctx.enter_context(tc.tile_pool(name="persist", bufs=1))
                XTb = persist.tile([128, CO, N], BF16)
                WQb = persist.tile([128, CO, DH], BF16)
                WKb = persist.tile([128, CO, DH], BF16)
                WVb = persist.tile([128, CO, DH], BF16)
                WPb = persist.tile([128, DO, OUTC], BF16)
                bias_sb = persist.tile([128, OUTC], F32)
                OT = persist.tile([128, DO, N], BF16)
                nbias = persist.tile([128, 1], F32)
                nc.vector.memset(nbias, EXP_BIAS)
                ones_bf = persist.tile([128, HD], BF16)
                nc.vector.memset(ones_bf, 1.0)

                ld = ctx.enter_context(tc.tile_pool(name="load", bufs=3))
                kqp = ctx.enter_context(tc.tile_pool(name="kq", bufs=2))
                vnp = ctx.enter_context(tc.tile_pool(name="vn", bufs=2))
                etp = ctx.enter_context(tc.tile_pool(name="et", bufs=4))
                rcpp = ctx.enter_context(tc.tile_pool(name="rcp", bufs=2))
                rbp = ctx.enter_context(tc.tile_pool(name="rb", bufs=2))
                osp = ctx.enter_context(tc.tile_pool(name="ostage", bufs=2))
                psS = ctx.enter_context(
                    tc.tile_pool(name="psS", bufs=2, space="PSUM")
                )
                psO = ctx.enter_context(
                    tc.tile_pool(name="psO", bufs=1, space="PSUM")
                )

                # ---- loads (DMA f32 stage -> bf16 cast) ----
                # PE warm-up burst keeps the HAM clock gate at full rate
                # through the DMA-paced load phase
                wps = psS.tile([128, QH], F32, tag="s")
                for wi in range(40):
                    nc.tensor.matmul(
                        wps[:, 0:512],
                        lhsT=XTb[:, 0, 0:128],
                        rhs=XTb[:, 0, 0:512],
                        start=(wi == 0),
                        stop=(wi == 39),
                    )
                for co in range(CO):
                    st = ld.tile([128, N], F32, tag="stage")
                    nc.sync.dma_start(out=st, in_=xT_e[co * 128 : (co + 1) * 128, :])
                    nc.vector.tensor_copy(XTb[:, co, :], st)
                for w_e, Wb in ((wqT_e, WQb), (wkT_e, WKb), (wvT_e, WVb)):
                    for co in range(CO):
                        st = ld.tile([128, N], F32, tag="stage")
                        nc.sync.dma_start(
                            out=st[:, :DH], in_=w_e[co * 128 : (co + 1) * 128, :]
                        )
                        nc.vector.tensor_copy(Wb[:, co, :], st[:, :DH])
                for do in range(DO):
                    st = ld.tile([128, N], F32, tag="stage")
                    nc.sync.dma_start(
                        out=st[:, :OUTC], in_=wpT_e[do * 128 : (do + 1) * 128, :]
                    )
                    nc.vector.tensor_copy(WPb[:, do, :], st[:, :OUTC])
                pb_bcast = bass.AP(
                    tensor=pb_e.tensor if hasattr(pb_e, "tensor") else pb_e,
                    offset=0,
                    ap=[[0, 128], [1, OUTC]],
                )
                nc.sync.dma_start(out=bias_sb, in_=pb_bcast)

                # ---- just-in-time QKV emitters (one PE chunk each) ----
                def kq_chunk(p, Wb, KTd, no):
                    ps = psS.tile([128, QH], F32, tag="s")
                    for co in range(CO):
                        nc.tensor.matmul(
                            ps[:, 0:512],
                            lhsT=Wb[:, co, p * 128 : (p + 1) * 128],
                            rhs=XTb[:, co, no * 512 : (no + 1) * 512],
                            start=(co == 0),
                            stop=(co == CO - 1),
                        )
                    nc.vector.tensor_copy(
                        KTd[:, no * 512 : (no + 1) * 512], ps[:, 0:512]
                    )

                def v_chunk(p, VNd, no):
                    ps = psS.tile([128, QH], F32, tag="s")
                    for co in range(CO):
                        nc.tensor.matmul(
                            ps[:, 0:128],
                            lhsT=XTb[:, co, no * 128 : (no + 1) * 128],
                            rhs=WVb[:, co, p * 128 : (p + 1) * 128],
                            start=(co == 0),
                            stop=(co == CO - 1),
                        )
                    nc.vector.tensor_copy(
                        VNd[:, no, :, 0:HD],
                        ps[:, 0:128].rearrange("p (h d) -> p h d", h=2),
                    )

                def claim_pair_tiles():
                    KTd = kqp.tile([128, N], BF16, tag="kt")
                    QTd = kqp.tile([128, N], BF16, tag="qt")
                    VNd = vnp.tile([128, KO, 2, HD + 1], BF16, tag="vn")
                    nc.vector.memset(VNd[:, :, :, HD], 1.0)
                    return KTd, QTd, VNd

                def qkv_work(p, tiles):
                    """List of closures, each one PE slot-chunk (<=~0.9us)."""
                    KTd, QTd, VNd = tiles
                    items = []
                    # K first (scores need K columns early), V interleaved
                    # (PV(ko) needs V block ko), Q half1 last.
                    kq_list = [(WKb, KTd, no) for no in range(4)] + [
                        (WQb, QTd, no) for no in range(4)
                    ]
                    for Wb, Td, no in kq_list:
                        items.append(
                            lambda Wb=Wb, Td=Td, no=no: kq_chunk(p, Wb, Td, no)
                        )
                    for no in range(KO):
                        items.append(lambda no=no: v_chunk(p, VNd, no))
                    return items

                # pair 0 QKV up front
                tiles_cur = claim_pair_tiles()
                for it in qkv_work(0, tiles_cur):
                    it()

                for p in range(NPAIR):
                    KTd, QTd, VNd = tiles_cur
                    tiles_next = None
                    nxt = []
                    if p + 1 < NPAIR:
                        tiles_next = claim_pair_tiles()
                        nxt = qkv_work(p + 1, tiles_next)
                    nxt_i = 0

                    for half in range(2):
                        pso_A = psO.tile([HD + 1, QH], F32, tag="oA", bufs=1)
                        pso_B = psO.tile([HD + 1, QH], F32, tag="oB", bufs=1)
                        prev = None  # (et_a, et_b) of previous ko
                        for ko in range(KO):
                            # one next-pair QKV chunk per slot, emitted first
                            # so its psum claim/release precedes the score
                            # tiles in the tag-"s" ring rotation
                            if nxt_i < len(nxt) and (half + ko) % 4 != 3:
                                nxt[nxt_i]()
                                nxt_i += 1
                            ps_a = psS.tile([128, QH], F32, tag="s")
                            ps_b = psS.tile([128, QH], F32, tag="s")
                            et_a = etp.tile([128, QH], BF16, tag="et")
                            et_b = etp.tile([128, QH], BF16, tag="et")
                            # scores head A (PE rows 0:64)
                            for qs in range(2):
                                qlo = half * QH + qs * 512
                                nc.tensor.matmul(
                                    ps_a[:, qs * 512 : (qs + 1) * 512],
                                    lhsT=KTd[0:HD, ko * 128 : (ko + 1) * 128],
                                    rhs=QTd[0:HD, qlo : qlo + 512],
                                    start=True,
                                    stop=True,
                                )
                            nc.scalar.activation(
                                out=et_a,
                                in_=ps_a,
                                func=mybir.ActivationFunctionType.Exp,
                                scale=SCALE,
                                bias=nbias,
                            )
                            # PV for previous ko, head A (lags one ko so the
                            # in-order PE never waits on this slot's exp)
                            if prev is not None:
                                for qs in range(2):
                                    nc.tensor.matmul(
                                        pso_A[:, qs * 512 : (qs + 1) * 512],
                                        lhsT=VNd[:, ko - 1, 0, :],
                                        rhs=prev[0][:, qs * 512 : (qs + 1) * 512],
                                        start=(ko - 1 == 0),
                                        stop=False,
                                    )
                            # scores head B (PE rows 64:128)
                            for qs in range(2):
                                qlo = half * QH + qs * 512
                                nc.tensor.matmul(
                                    ps_b[:, qs * 512 : (qs + 1) * 512],
                                    lhsT=KTd[HD:128, ko * 128 : (ko + 1) * 128],
                                    rhs=QTd[HD:128, qlo : qlo + 512],
                                    start=True,
                                    stop=True,
                                )
                            nc.scalar.activation(
                                out=et_b,
                                in_=ps_b,
                                func=mybir.ActivationFunctionType.Exp,
                                scale=SCALE,
                                bias=nbias,
                            )
                            if prev is not None:
                                for qs in range(2):
                                    nc.tensor.matmul(
                                        pso_B[:, qs * 512 : (qs + 1) * 512],
                                        lhsT=VNd[:, ko - 1, 1, :],
                                        rhs=prev[1][:, qs * 512 : (qs + 1) * 512],
                                        start=(ko - 1 == 0),
                                        stop=False,
                                    )
                            prev = (et_a, et_b)
                        # last ko's PV
                        for hidx, (pso, et) in enumerate(
                            ((pso_A, prev[0]), (pso_B, prev[1]))
                        ):
                            for qs in range(2):
                                nc.tensor.matmul(
                                    pso[:, qs * 512 : (qs + 1) * 512],
                                    lhsT=VNd[:, KO - 1, hidx, :],
                                    rhs=et[:, qs * 512 : (qs + 1) * 512],
                                    start=False,
                                    stop=True,
                                )

                        # ---- normalization per head ----
                        # rcp the denominator row (f32), cast to bf16, PE
                        # outer-product broadcast down 64 partitions, then
                        # one DVE multiply (SBUF numerators x PSUM bcast)
                        for hidx, pso in enumerate((pso_A, pso_B)):
                            rcp_row = rcpp.tile([128, QH], F32, tag="rr")
                            nc.vector.reciprocal(
                                rcp_row[HD : HD + 1, :],
                                pso[HD : HD + 1, :],
                            )
                            rcpb = rbp.tile([128, QH], BF16, tag="rb")
                            nc.vector.tensor_copy(
                                rcpb[HD : HD + 1, :], rcp_row[HD : HD + 1, :]
                            )
                            osb = rbp.tile([HD, QH], BF16, tag="osb")
                            nc.vector.tensor_copy(osb, pso[0:HD, :])
                            bc = psS.tile([128, QH], F32, tag="s")
                            for qs in range(2):
                                nc.tensor.matmul(
                                    bc[0:HD, qs * 512 : (qs + 1) * 512],
                                    lhsT=ones_bf[HD : HD + 1, :],
                                    rhs=rcpb[HD : HD + 1, qs * 512 : (qs + 1) * 512],
                                    start=True,
                                    stop=True,
                                )
                            row = hidx * HD
                            nc.vector.tensor_tensor(
                                OT[row : row + HD, p, half * QH : (half + 1) * QH],
                                osb,
                                bc[0:HD, :],
                                mybir.AluOpType.mult,
                            )
                    # drain any leftover next-pair QKV chunks
                    while nxt_i < len(nxt):
                        nxt[nxt_i]()
                        nxt_i += 1
                    tiles_cur = tiles_next

                # ---- output projection (+ bias) ----
                for no in range(N // 128):
                    st = osp.tile([128, OUTC], F32, tag="ostage")
                    for oc in range(2):
                        ps = psS.tile([128, QH], F32, tag="s")
                        for ci in range(DO):
                            nc.tensor.matmul(
                                ps[:, 0:512],
                                lhsT=OT[:, ci, no * 128 : (no + 1) * 128],
                                rhs=WPb[:, ci, oc * 512 : (oc + 1) * 512],
                                start=(ci == 0),
                                stop=(ci == DO - 1),
                            )
                        nc.vector.tensor_tensor(
                            st[:, oc * 512 : (oc + 1) * 512],
                            ps[:, 0:512],
                            bias_sb[:, oc * 512 : (oc + 1) * 512],
                            mybir.AluOpType.add,
                        )
                    nc.sync.dma_start(out=out_e[no * 128 : (no + 1) * 128, :], in_=st)

        if R == 1:
            body()
        else:
            with tc.For_i(0, R, 1) as iv:
                body(iv)

    _split_multi_waits(nc)
    return nc


def shard_inputs(x, qkv_w, proj_w, proj_b):
    in_maps = []
    for i in range(8):
        b, g = i // 2, i % 2
        sl = slice(g * DH, (g + 1) * DH)
        xT = np.ascontiguousarray(x[b].T).astype(np.float32)
        wqT = np.ascontiguousarray(qkv_w[0 * C : 1 * C][sl, :].T).astype(np.float32)
        wkT = np.ascontiguousarray(qkv_w[1 * C : 2 * C][sl, :].T).astype(np.float32)
        wvT = np.ascontiguousarray(qkv_w[2 * C : 3 * C][sl, :].T).astype(np.float32)
        wpT = np.ascontiguousarray(proj_w[:, sl].T).astype(np.float32)
        pb = (proj_b if g == 0 else np.zeros_like(proj_b)).reshape(1, -1)
        in_maps.append(
            {
                "xT": xT,
                "wqT": wqT,
                "wkT": wkT,
                "wvT": wvT,
                "wpT": wpT,
                "pb": np.ascontiguousarray(pb).astype(np.float32),
            }
        )
    return in_maps


_CACHED_NC = None


def kernel(x, qkv_w, proj_w, proj_b):
    """Full inputs in, full output out. Shards over 8 NeuronCores."""
    global _CACHED_NC
    x = np.asarray(x, dtype=np.float32)
    qkv_w = np.asarray(qkv_w, dtype=np.float32)
    proj_w = np.asarray(proj_w, dtype=np.float32)
    proj_b = np.asarray(proj_b, dtype=np.float32)

    if _CACHED_NC is None:
        _CACHED_NC = build_attention_nc(R=1)
    nc = _CACHED_NC

    in_maps = shard_inputs(x, qkv_w, proj_w, proj_b)
    res = run_bass_kernel_spmd(nc, in_maps, core_ids=list(range(8)))
    out = np.empty((B, N, OUTC), dtype=np.float32)
    for b in range(B):
        out[b] = res.results[2 * b]["out"] + res.results[2 * b + 1]["out"]
    return out


# revision 21
# speedup vs baseline: 1.2805x; 1.2805x over previous
"""Multi-head attention (B=4, N=2048, C=1024, H=16, HD=64) on 8 TRN2 NeuronCores.

Self-contained: takes the FULL unsharded inputs of the reference problem,
shards across 8 cores, runs a Bass/Tile kernel SPMD, and reassembles the
full output.

Sharding (tensor-parallel over heads x data-parallel over batch):
  core i -> batch b = i//2, head-group g = i%2 (8 of the 16 heads).
  Unsharding sums the two partial output projections per batch.

v2 pipeline (per core, heads processed in 4 pairs sharing a 128-row dslot):
  per pair p (head A=2p at partitions 0:64, B=2p+1 at 64:128):
    KTp/QTp [128d, N] and VNp [128k, ko, 2, 65] computed just-in-time;
    pair p+1's QKV matmul groups are sprinkled one-per-slot into pair p's
    32 (half, ko) attention slots so the PE never idles the ScalarE.
    per slot: S^T tiles [128k, 1024q] for A and B (the two heads' K=64
      matmuls land in disjoint PE row-groups via their base partitions);
      exp on ScalarE (scale=1/8, bias=-5 keeps E finite; the constant
      shift cancels in softmax) -> ET bf16; PV lags one ko so the
      in-order PE never waits on the exp it just requested.
      PV keeps V stationary (LDWEIGHTS amortized over 512-wide ET
      streams): psO[65, 1024] += VN_aug.T @ ET -- the ones column
      (row 64) accumulates the softmax denominators.
    norm per (head, half): reciprocal_approx_fast on the den row,
      GPSIMD partition_broadcast down 64 partitions, one DVE multiply
      -> OT[d, n] (already transposed for the projection; no PE
      transposes anywhere).
  out [N, OUTC] = OT.T @ wpT + pb  (partial: this head-group's channels)
"""

import sys

sys.path.insert(0, "/opt/trn_rl_repo")

from contextlib import ExitStack

import numpy as np

import concourse.bass as bass
import concourse.tile as tile
from concourse import mybir
from concourse.bass_utils import run_bass_kernel_spmd

F32 = mybir.dt.float32
BF16 = mybir.dt.bfloat16

B, N, C, H = 4, 2048, 1024, 16
HH = H // 2          # heads per core
HD = C // H          # head dim
DH = HH * HD         # attention channels per core
OUTC = C
NPAIR = HH // 2      # head pairs per core (share a 128-row dslot)
KO = N // 128        # 16 key blocks
SCALE = float(HD) ** -0.5
EXP_BIAS = -5.0      # exp(s*scale + bias); cancels in softmax normalization


def _split_multi_waits(nc, max_waits=1):
    """The pinned walrus build rejects >1 SyncWait on engine CTRL
    instructions; move extra waits onto preceding NOPs."""
    n_split = 0
    for bb in nc.main_func.blocks:
        insts = list(bb.instructions)
        new_insts = []
        changed = False
        for ins in insts:
            si = getattr(ins, "sync_info", None)
            nm = type(ins).__name__
            is_dma = "Dma" in nm or "TensorLoad" in nm or "TensorSave" in nm
            if si is not None and not is_dma:
                waits = list(si.on_wait)
                if len(waits) > max_waits:
                    head, tail = waits[:-max_waits], waits[-max_waits:]
                    for i in range(0, len(head), max_waits):
                        new_insts.append(
                            mybir.InstNoOp(
                                name=f"{ins.name}-ws{n_split}-{i}",
                                engine=ins.engine,
                                sync_info=mybir.SyncInfo(
                                    on_wait=head[i : i + max_waits], on_update=[]
                                ),
                                bass_nofuse=True,
                            )
                        )
                    ins.sync_info = mybir.SyncInfo(
                        on_wait=tail, on_update=list(si.on_update)
                    )
                    n_split += 1
                    changed = True
            new_insts.append(ins)
        if changed:
            bb.instructions = new_insts
    return n_split


def build_attention_nc(R=1):
    CO = C // 128        # 8 contraction blocks for qkv projections
    DO = DH // 128       # 4 dslots
    QH = N // 2          # 1024-query half

    nc = bass.Bass()
    xT_e = nc.declare_dram_parameter("xT", [C, N], F32, isOutput=False)
    wqT_e = nc.declare_dram_parameter("wqT", [C, DH], F32, isOutput=False)
    wkT_e = nc.declare_dram_parameter("wkT", [C, DH], F32, isOutput=False)
    wvT_e = nc.declare_dram_parameter("wvT", [C, DH], F32, isOutput=False)
    wpT_e = nc.declare_dram_parameter("wpT", [DH, OUTC], F32, isOutput=False)
    pb_e = nc.declare_dram_parameter("pb", [1, OUTC], F32, isOutput=False)
    out_e = nc.declare_dram_parameter("out", [N, OUTC], F32, isOutput=True)

    with tile.TileContext(nc) as tc:

        def body(_iv=None):
            with ExitStack() as ctx:
                persist = ctx.enter_context(tc.tile_pool(name="persist", bufs=1))
                XTb = persist.tile([128, CO, N], BF16)
                WQb = persist.tile([128, CO, DH], BF16)
                WKb = persist.tile([128, CO, DH], BF16)
                WVb = persist.tile([128, CO, DH], BF16)
                WPb = persist.tile([128, DO, OUTC], BF16)
                bias_sb = persist.tile([128, OUTC], F32)
                OT = persist.tile([128, DO, N], BF16)
                nbias = persist.tile([128, 1], F32)
                nc.vector.memset(nbias, EXP_BIAS)
                ones_bf = persist.tile([128, HD], BF16)
                nc.vector.memset(ones_bf, 1.0)
                # softmax denominators, DMA-gathered from partition 64 of
                # each psO tile onto one lane per (head, half) so a single
                # multi-lane reciprocal covers a whole pair
                # pair p's 4 rows live at partitions 32p..32p+3 (engine ops
                # need 32-aligned partition bases)
                dall = persist.tile([128, QH], F32)
                drcp = persist.tile([128, QH], F32)
                dbf = persist.tile([128, QH], BF16)

                ld = ctx.enter_context(tc.tile_pool(name="load", bufs=3))
                kqp = ctx.enter_context(tc.tile_pool(name="kq", bufs=2))
                vnp = ctx.enter_context(tc.tile_pool(name="vn", bufs=2))
                etp = ctx.enter_context(tc.tile_pool(name="et", bufs=4))
                rcpp = ctx.enter_context(tc.tile_pool(name="rcp", bufs=2))
                rbp = ctx.enter_context(tc.tile_pool(name="rb", bufs=2))
                osp = ctx.enter_context(tc.tile_pool(name="ostage", bufs=2))
                psS = ctx.enter_context(
                    tc.tile_pool(name="psS", bufs=2, space="PSUM")
                )
                psO = ctx.enter_context(
                    tc.tile_pool(name="psO", bufs=1, space="PSUM")
                )

                # ---- loads (DMA f32 stage -> bf16 cast) ----
                # PE warm-up burst keeps the HAM clock gate at full rate
                # through the DMA-paced load phase
                wps = psS.tile([128, QH], F32, tag="s")
                for wi in range(40):
                    nc.tensor.matmul(
                        wps[:, 0:512],
                        lhsT=XTb[:, 0, 0:128],
                        rhs=XTb[:, 0, 0:512],
                        start=(wi == 0),
                        stop=(wi == 39),
                    )
                for co in range(CO):
                    st = ld.tile([128, N], F32, tag="stage")
                    nc.sync.dma_start(out=st, in_=xT_e[co * 128 : (co + 1) * 128, :])
                    nc.vector.tensor_copy(XTb[:, co, :], st)
                for w_e, Wb in ((wqT_e, WQb), (wkT_e, WKb), (wvT_e, WVb)):
                    for co in range(CO):
                        st = ld.tile([128, N], F32, tag="stage")
                        nc.sync.dma_start(
                            out=st[:, :DH], in_=w_e[co * 128 : (co + 1) * 128, :]
                        )
                        nc.vector.tensor_copy(Wb[:, co, :], st[:, :DH])
                for do in range(DO):
                    st = ld.tile([128, N], F32, tag="stage")
                    nc.sync.dma_start(
                        out=st[:, :OUTC], in_=wpT_e[do * 128 : (do + 1) * 128, :]
                    )
                    nc.vector.tensor_copy(WPb[:, do, :], st[:, :OUTC])
                pb_bcast = bass.AP(
                    tensor=pb_e.tensor if hasattr(pb_e, "tensor") else pb_e,
                    offset=0,
                    ap=[[0, 128], [1, OUTC]],
                )
                nc.sync.dma_start(out=bias_sb, in_=pb_bcast)

                # ---- just-in-time QKV emitters (one PE chunk each) ----
                def kq_chunk(p, Wb, KTd, no):
                    ps = psS.tile([128, QH], F32, tag="s")
                    for co in range(CO):
                        nc.tensor.matmul(
                            ps[:, 0:512],
                            lhsT=Wb[:, co, p * 128 : (p + 1) * 128],
                            rhs=XTb[:, co, no * 512 : (no + 1) * 512],
                            start=(co == 0),
                            stop=(co == CO - 1),
                        )
                    nc.vector.tensor_copy(
                        KTd[:, no * 512 : (no + 1) * 512], ps[:, 0:512]
                    )

                def v_chunk(p, VNd, no):
                    ps = psS.tile([128, QH], F32, tag="s")
                    for co in range(CO):
                        nc.tensor.matmul(
                            ps[:, 0:128],
                            lhsT=XTb[:, co, no * 128 : (no + 1) * 128],
                            rhs=WVb[:, co, p * 128 : (p + 1) * 128],
                            start=(co == 0),
                            stop=(co == CO - 1),
                        )
                    nc.vector.tensor_copy(
                        VNd[:, no, :, 0:HD],
                        ps[:, 0:128].rearrange("p (h d) -> p h d", h=2),
                    )

                def claim_pair_tiles():
                    KTd = kqp.tile([128, N], BF16, tag="kt")
                    QTd = kqp.tile([128, N], BF16, tag="qt")
                    VNd = vnp.tile([128, KO, 2, HD + 1], BF16, tag="vn")
                    nc.vector.memset(VNd[:, :, :, HD], 1.0)
                    return KTd, QTd, VNd

                def qkv_work(p, tiles):
                    """List of closures, each one PE slot-chunk (<=~0.9us)."""
                    KTd, QTd, VNd = tiles
                    items = []
                    # K first (scores need K columns early), V interleaved
                    # (PV(ko) needs V block ko), Q half1 last.
                    kq_list = [(WKb, KTd, no) for no in range(4)] + [
                        (WQb, QTd, no) for no in range(4)
                    ]
                    for Wb, Td, no in kq_list:
                        items.append(
                            lambda Wb=Wb, Td=Td, no=no: kq_chunk(p, Wb, Td, no)
                        )
                    for no in range(KO):
                        items.append(lambda no=no: v_chunk(p, VNd, no))
                    return items

                # pair 0 QKV up front
                tiles_cur = claim_pair_tiles()
                for it in qkv_work(0, tiles_cur):
                    it()

                stash = []

                for p in range(NPAIR):
                    KTd, QTd, VNd = tiles_cur
                    tiles_next = None
                    nxt = []
                    if p + 1 < NPAIR:
                        tiles_next = claim_pair_tiles()
                        nxt = qkv_work(p + 1, tiles_next)
                    nxt_i = 0

                    for half in range(2):
                        pso_A = psO.tile([HD + 1, QH], F32, tag="oA", bufs=1)
                        pso_B = psO.tile([HD + 1, QH], F32, tag="oB", bufs=1)
                        prev = None  # (et_a, et_b) of previous ko
                        for ko in range(KO):
                            # one next-pair QKV chunk per slot, emitted first
                            # so its psum claim/release precedes the score
                            # tiles in the tag-"s" ring rotation
                            if nxt_i < len(nxt) and (half + ko) % 4 != 3:
                                nxt[nxt_i]()
                                nxt_i += 1
                            ps_a = psS.tile([128, QH], F32, tag="s")
                            ps_b = psS.tile([128, QH], F32, tag="s")
                            et_a = etp.tile([128, QH], BF16, tag="et")
                            et_b = etp.tile([128, QH], BF16, tag="et")
                            # scores head A (PE rows 0:64)
                            for qs in range(2):
                                qlo = half * QH + qs * 512
                                nc.tensor.matmul(
                                    ps_a[:, qs * 512 : (qs + 1) * 512],
                                    lhsT=KTd[0:HD, ko * 128 : (ko + 1) * 128],
                                    rhs=QTd[0:HD, qlo : qlo + 512],
                                    start=True,
                                    stop=True,
                                )
                            nc.scalar.activation(
                                out=et_a,
                                in_=ps_a,
                                func=mybir.ActivationFunctionType.Exp,
                                scale=SCALE,
                                bias=nbias,
                            )
                            # PV for previous ko, head A (lags one ko so the
                            # in-order PE never waits on this slot's exp)
                            if prev is not None:
                                for qs in range(2):
                                    nc.tensor.matmul(
                                        pso_A[:, qs * 512 : (qs + 1) * 512],
                                        lhsT=VNd[:, ko - 1, 0, :],
                                        rhs=prev[0][:, qs * 512 : (qs + 1) * 512],
                                        start=(ko - 1 == 0),
                                        stop=False,
                                    )
                            # scores head B (PE rows 64:128)
                            for qs in range(2):
                                qlo = half * QH + qs * 512
                                nc.tensor.matmul(
                                    ps_b[:, qs * 512 : (qs + 1) * 512],
                                    lhsT=KTd[HD:128, ko * 128 : (ko + 1) * 128],
                                    rhs=QTd[HD:128, qlo : qlo + 512],
                                    start=True,
                                    stop=True,
                                )
                            nc.scalar.activation(
                                out=et_b,
                                in_=ps_b,
                                func=mybir.ActivationFunctionType.Exp,
                                scale=SCALE,
                                bias=nbias,
                            )
                            if prev is not None:
                                for qs in range(2):
                                    nc.tensor.matmul(
                                        pso_B[:, qs * 512 : (qs + 1) * 512],
                                        lhsT=VNd[:, ko - 1, 1, :],
                                        rhs=prev[1][:, qs * 512 : (qs + 1) * 512],
                                        start=(ko - 1 == 0),
                                        stop=False,
                                    )
                            prev = (et_a, et_b)
                        # last ko's PV
                        for hidx, (pso, et) in enumerate(
                            ((pso_A, prev[0]), (pso_B, prev[1]))
                        ):
                            for qs in range(2):
                                nc.tensor.matmul(
                                    pso[:, qs * 512 : (qs + 1) * 512],
                                    lhsT=VNd[:, KO - 1, hidx, :],
                                    rhs=et[:, qs * 512 : (qs + 1) * 512],
                                    start=False,
                                    stop=True,
                                )

                        # ---- stage psO to SBUF (frees banks fast) ----
                        for hidx, pso in enumerate((pso_A, pso_B)):
                            osb = rbp.tile([HD + 1, QH], F32, tag="osb", bufs=4)
                            nc.vector.tensor_copy(osb, pso)
                            idx = p * 32 + half * 2 + hidx
                            nc.sync.dma_start(
                                out=dall[idx : idx + 1, :], in_=osb[HD : HD + 1, :]
                            )
                            stash.append((osb, idx, p, half, hidx))

                    # ---- normalization for this pair (off critical path):
                    # one 4-lane reciprocal, bf16 cast, PE broadcast, DVE mult
                    nc.vector.reciprocal(
                        drcp[p * 32 : p * 32 + 4, :], dall[p * 32 : p * 32 + 4, :]
                    )
                    nc.vector.tensor_copy(
                        dbf[p * 32 : p * 32 + 4, :], drcp[p * 32 : p * 32 + 4, :]
                    )
                    for osb, idx, pp, hf, hidx in stash:
                        dbc = rbp.tile([1, QH], BF16, tag="dbc", bufs=4)
                        nc.sync.dma_start(out=dbc, in_=dbf[idx : idx + 1, :])
                        bc = psO.tile(
                            [HD + 1, QH], F32, tag="oA" if hidx == 0 else "oB"
                        )
                        for qs in range(2):
                            nc.tensor.matmul(
                                bc[0:HD, qs * 512 : (qs + 1) * 512],
                                lhsT=ones_bf[0:1, :],
                                rhs=dbc[:, qs * 512 : (qs + 1) * 512],
                                start=True,
                                stop=True,
                            )
                        row = hidx * HD
                        nc.vector.tensor_tensor(
                            OT[row : row + HD, pp, hf * QH : (hf + 1) * QH],
                            osb[0:HD, :],
                            bc[0:HD, :],
                            mybir.AluOpType.mult,
                        )
                    stash = []
                    # drain any leftover next-pair QKV chunks
                    while nxt_i < len(nxt):
                        nxt[nxt_i]()
                        nxt_i += 1
                    tiles_cur = tiles_next

                # ---- output projection (+ bias) ----
                for no in range(N // 128):
                    st = osp.tile([128, OUTC], F32, tag="ostage")
                    for oc in range(2):
                        ps = psS.tile([128, QH], F32, tag="s")
                        for ci in range(DO):
                            nc.tensor.matmul(
                                ps[:, 0:512],
                                lhsT=OT[:, ci, no * 128 : (no + 1) * 128],
                                rhs=WPb[:, ci, oc * 512 : (oc + 1) * 512],
                                start=(ci == 0),
                                stop=(ci == DO - 1),
                            )
                        nc.vector.tensor_tensor(
                            st[:, oc * 512 : (oc + 1) * 512],
                            ps[:, 0:512],
                            bias_sb[:, oc * 512 : (oc + 1) * 512],
                            mybir.AluOpType.add,
                        )
                    nc.sync.dma_start(out=out_e[no * 128 : (no + 1) * 128, :], in_=st)

        if R == 1:
            body()
        else:
            with tc.For_i(0, R, 1) as iv:
                body(iv)

    _split_multi_waits(nc)
    return nc


def shard_inputs(x, qkv_w, proj_w, proj_b):
    in_maps = []
    for i in range(8):
        b, g = i // 2, i % 2
        sl = slice(g * DH, (g + 1) * DH)
        xT = np.ascontiguousarray(x[b].T).astype(np.float32)
        wqT = np.ascontiguousarray(qkv_w[0 * C : 1 * C][sl, :].T).astype(np.float32)
        wkT = np.ascontiguousarray(qkv_w[1 * C : 2 * C][sl, :].T).astype(np.float32)
        wvT = np.ascontiguousarray(qkv_w[2 * C : 3 * C][sl, :].T).astype(np.float32)
        wpT = np.ascontiguousarray(proj_w[:, sl].T).astype(np.float32)
        pb = (proj_b if g == 0 else np.zeros_like(proj_b)).reshape(1, -1)
        in_maps.append(
            {
                "xT": xT,
                "wqT": wqT,
                "wkT": wkT,
                "wvT": wvT,
                "wpT": wpT,
                "pb": np.ascontiguousarray(pb).astype(np.float32),
            }
        )
    return in_maps


_CACHED_NC = None


def kernel(x, qkv_w, proj_w, proj_b):
    """Full inputs in, full output out. Shards over 8 NeuronCores."""
    global _CACHED_NC
    x = np.asarray(x, dtype=np.float32)
    qkv_w = np.asarray(qkv_w, dtype=np.float32)
    proj_w = np.asarray(proj_w, dtype=np.float32)
    proj_b = np.asarray(proj_b, dtype=np.float32)

    if _CACHED_NC is None:
        _CACHED_NC = build_attention_nc(R=1)
    nc = _CACHED_NC

    in_maps = shard_inputs(x, qkv_w, proj_w, proj_b)
    res = run_bass_kernel_spmd(nc, in_maps, core_ids=list(range(8)))
    out = np.empty((B, N, OUTC), dtype=np.float32)
    for b in range(B):
        out[b] = res.results[2 * b]["out"] + res.results[2 * b + 1]["out"]
    return out
